# revision 13
# baseline (speedup 1.0000x reference)
"""DEDICOM decoder edge scoring on 8 TRN2 NeuronCores.

scores[e] = (z[src_e] * d) @ R @ (z[dst_e] * d)  for 1M edges.

The per-edge row gather is the bottleneck: SWDGE descriptor generation
costs ~2-3 ns/row even with 4 queues pipelining across Q7 core pairs.
So only the DST side uses dma_gather; the SRC side is computed with
TensorE one-hot selection matmuls, which need no descriptors:

  - the host groups edges into (dst-half, src-block) cells and deals
    each cell's edges round-robin across the 8 cores, so every core has
    the IDENTICAL padded layout (all cores share one SPMD program).
    Within a cell, edges are sorted by dst (helps HBM locality of the
    gather).  Host emits, per 512-edge slice, the piece list
    (src_block, lo, hi) plus a one-hot matrix
    onehot[i, slot] = (src_slot mod 128 == i)  (zero for pad slots).
  - device builds Y = z @ M (M = (d (x) d) * R) in bf16, resident in
    SBUF as [128, 391, 128] (Y block b, row r, at [r, b, :]).
  - per 2048-edge chunk: zdT = transpose-mode dma_gather of z[dst]
    (bf16 256B rows -> [128 feat, 2048 edge] columns), round-robin on
    4 SWDGE queues.  Per 512-slice:
      mzs  = Y_b^T @ onehot_piece   (TensorE -> PSUM, per piece)
      prod = mzs * zdT              (DVE, bf16 out)
      sc   = ones^T @ prod          (TensorE -> PSUM [1, 512])
      ACT copies sc to SBUF; Sync DMAs each chunk's scores to DRAM.
  - dst indices are int16, so z is addressed in halves (< / >= 32000).
"""
import numpy as np
import ml_dtypes
import concourse.bacc as bacc
import concourse.mybir as mybir
from concourse.tile import TileContext
from concourse.bass_utils import run_bass_kernel_spmd

N_CORES = 8
N_NODES = 50000
D = 128
HALF = 32000          # int16-safe dst table split point
CHUNK = 4096          # edges per dma_gather call
SLICE = 512           # edges per compute slice (1 PSUM bank)
NQ = 4                # SWDGE queues (queue q -> Q7 core pair q)
NBLK = (N_NODES + 127) // 128             # 391 src blocks
NPAD = NBLK * 128                         # 50048


def _build_program(nchunks_per_bucket, pieces):
    ntot = sum(nchunks_per_bucket) * CHUNK
    nc = bacc.Bacc("TRN2", num_devices=N_CORES, num_swdge_queues=NQ)
    zb = nc.declare_dram_parameter("zb", [NPAD, D], mybir.dt.bfloat16, isOutput=False)
    zbT = nc.declare_dram_parameter("zbT", [128, NPAD], mybir.dt.bfloat16, isOutput=False)
    R = nc.declare_dram_parameter("R", [D, D], mybir.dt.float32, isOutput=False)
    dr = nc.declare_dram_parameter("dr", [1, D], mybir.dt.float32, isOutput=False)
    ones = nc.declare_dram_parameter("ones", [128, 1], mybir.dt.bfloat16, isOutput=False)
    onehot = nc.declare_dram_parameter("onehot", [128, ntot], mybir.dt.bfloat16, isOutput=False)
    idst = nc.declare_dram_parameter("idst", [128, ntot // 16], mybir.dt.int16, isOutput=False)
    scores = nc.declare_dram_parameter("scores", [1, ntot], mybir.dt.float32, isOutput=True)

    with TileContext(nc) as tc:
        with (
            tc.tile_pool(name="const", bufs=1) as constp,
            tc.tile_pool(name="yres", bufs=1) as yresp,
            tc.tile_pool(name="zslab", bufs=1) as zslabp,
            tc.tile_pool(name="ybuild", bufs=2, space="PSUM") as ybps,
            tc.tile_pool(name="idxp", bufs=1) as idxp,
            tc.tile_pool(name="gdst", bufs=2) as gdstp,
            tc.tile_pool(name="zt", bufs=2) as ztp,
            tc.tile_pool(name="ohp", bufs=2) as ohp,
            tc.tile_pool(name="mzs", bufs=3, space="PSUM") as mzsp,
            tc.tile_pool(name="prod", bufs=2) as prodp,
            tc.tile_pool(name="scps", bufs=3, space="PSUM") as scpsp,
            tc.tile_pool(name="scsb", bufs=1) as scsbp,
        ):
            # ---- constants: M = (d (x) d) * R in bf16 ----
            R_sb = constp.tile([128, D], mybir.dt.float32)
            nc.sync.dma_start(out=R_sb[:], in_=R[:])
            dr_sb = constp.tile([1, D], mybir.dt.float32)
            nc.sync.dma_start(out=dr_sb[:], in_=dr[:])
            ones_sb = constp.tile([128, 1], mybir.dt.bfloat16)
            nc.sync.dma_start(out=ones_sb[:], in_=ones[:])
            DRps = ybps.tile([128, 4, 128], mybir.dt.float32, tag="yps")
            nc.tensor.matmul(out=DRps[:, 0, :], lhsT=dr_sb[:], rhs=dr_sb[:],
                             start=True, stop=True)
            Mb = constp.tile([128, D], mybir.dt.bfloat16)
            nc.vector.tensor_tensor(out=Mb[:], in0=R_sb[:], in1=DRps[:, 0, :],
                                    op=mybir.AluOpType.mult)

            # ---- indices ----
            idst_sb = idxp.tile([128, ntot // 16], mybir.dt.int16)
            nc.sync.dma_start(out=idst_sb[:], in_=idst[:])

            # ---- Y = z @ M resident in SBUF, blocked [128, NBLK, 128] ----
            Yres = yresp.tile([128, NBLK, 128], mybir.dt.bfloat16)
            for s0 in range(0, NBLK, 32):          # 32-block slabs of zbT
                sw = min(32, NBLK - s0)
                zsb = zslabp.tile([128, 32 * 128], mybir.dt.bfloat16, tag="zslab")
                nc.sync.dma_start(
                    out=zsb[:, :sw * 128],
                    in_=zbT[:, s0 * 128:(s0 + sw) * 128])
                i = 0
                while i < sw:
                    w = min(4, sw - i)
                    yps = ybps.tile([128, 4, 128], mybir.dt.float32, tag="yps")
                    for j in range(w):
                        nc.tensor.matmul(
                            out=yps[:, j, :],
                            lhsT=zsb[:, (i + j) * 128:(i + j + 1) * 128],
                            rhs=Mb[:], start=True, stop=True)
                    nc.scalar.copy(out=Yres[:, s0 + i:s0 + i + w, :], in_=yps[:, :w, :])
                    i += w

            # ---- main loop ----
            nchunks = ntot // CHUNK
            spc = CHUNK // SLICE
            LA = 2          # oh-load lookahead (chunks)
            oh_tiles = {}
            g_tiles = {}    # chunk -> (gather tile, sub-chunk offset)

            def load_oh(k):
                oh = ohp.tile([128, CHUNK], mybir.dt.bfloat16, tag="oh")
                nc.scalar.dma_start(out=oh[:], in_=onehot[:, k * CHUNK:(k + 1) * CHUNK])
                oh_tiles[k] = oh

            # pair chunks within each bucket: one dma_gather serves 2 chunks
            # (fewer calls amortize the ~3us per-call Q7 fixed cost)
            gq = [0]

            def gather(k0, npair):
                ng = npair * CHUNK
                g = gdstp.tile([128, 2 * CHUNK // 128, D], mybir.dt.bfloat16, tag="g")
                b = 0 if k0 < nchunks_per_bucket[0] else 1
                dst_t = zb[:, :] if b == 0 else zb[HALF:, :]
                c16 = k0 * (CHUNK // 16)
                nc.gpsimd.dma_gather(
                    g[:, :npair * (CHUNK // 128), :], dst_t,
                    idst_sb[:, c16:c16 + ng // 16],
                    ng, ng, D, single_packet=False, queue_num=gq[0] % NQ)
                gq[0] += 1
                for i in range(npair):
                    g_tiles[k0 + i] = (g, i)

            pairs = []
            for lo, hi in ((0, nchunks_per_bucket[0]), (nchunks_per_bucket[0], nchunks)):
                k = lo
                while k < hi:
                    npair = min(2, hi - k)
                    pairs.append((k, npair))
                    k += npair
            GLA = 1         # gather lookahead (pairs beyond current)
            for (k0, npair) in pairs[:GLA + 1]:
                gather(k0, npair)
            pi = GLA + 1    # next pair to issue

            for k in range(min(LA, ntot // CHUNK)):
                load_oh(k)
            for k in range(nchunks):
                g, gi = g_tiles.pop(k)
                if pi < len(pairs) and k == pairs[pi - GLA - 1][0]:
                    gather(*pairs[pi])
                    pi += 1
                # rows -> columns: zdTt[f, c, p] = g[p, c, f]; edge e=(c,p)
                zdTt = ztp.tile([128, CHUNK // 128, D], mybir.dt.bfloat16, tag="zt")
                nc.sync.dma_start_transpose(
                    out=zdTt[:],
                    in_=g[:, gi * (CHUNK // 128):(gi + 1) * (CHUNK // 128), :])
                zdT = zdTt[:, :, :].rearrange("f c p -> f (c p)")
                if k + LA < nchunks:
                    load_oh(k + LA)
                oh = oh_tiles.pop(k)
                sc_sb = scsbp.tile([1, CHUNK], mybir.dt.float32, tag="scsb")
                for j in range(spc):
                    si = k * spc + j
                    mzs = mzsp.tile([128, SLICE], mybir.dt.float32, tag="mzs")
                    for (blk, lo, hi) in pieces[si]:
                        nc.tensor.matmul(
                            out=mzs[:, lo:hi], lhsT=Yres[:, blk, :],
                            rhs=oh[:, j * SLICE + lo:j * SLICE + hi],
                            start=True, stop=True)
                    prod = prodp.tile([128, SLICE], mybir.dt.bfloat16, tag="prod")
                    nc.vector.tensor_tensor(
                        out=prod[:], in0=mzs[:],
                        in1=zdT[:, j * SLICE:(j + 1) * SLICE],
                        op=mybir.AluOpType.mult)
                    scp = scpsp.tile([1, SLICE], mybir.dt.float32, tag="scps")
                    nc.tensor.matmul(out=scp[:], lhsT=ones_sb[:], rhs=prod[:],
                                     start=True, stop=True)
                    nc.scalar.copy(out=sc_sb[:, j * SLICE:(j + 1) * SLICE], in_=scp[:])
                nc.scalar.dma_start(
                    out=scores[:, k * CHUNK:(k + 1) * CHUNK], in_=sc_sb[:])
    nc.compile()
    return nc


def _prepare(inputs):
    z = np.asarray(inputs["z"], dtype=np.float32)
    R = np.ascontiguousarray(np.asarray(inputs["R"], dtype=np.float32))
    Dm = np.asarray(inputs["D"], dtype=np.float32)
    edge_index = np.asarray(inputs["edge_index"])
    rel = int(np.asarray(inputs["relation_idx"]))
    dr = np.ascontiguousarray(Dm[rel:rel + 1, :])
    zb = np.zeros((NPAD, D), dtype=ml_dtypes.bfloat16)
    zb[:N_NODES] = z.astype(ml_dtypes.bfloat16)
    zbT = np.ascontiguousarray(zb.T)

    B = edge_index.shape[1]
    src = edge_index[0].astype(np.int64)
    dst = edge_index[1].astype(np.int64)

    # ---- cell assignment: (dst-half, src-block), dealt across cores ----
    h = (dst >= HALF).astype(np.int64)
    blk = src // 128
    cell = h * NBLK + blk                     # 0 .. 2*NBLK-1
    order = np.lexsort((dst, cell))           # cell-major, dst-sorted inside
    cell_sorted = cell[order]
    # per-cell counts and starts
    cell_counts = np.bincount(cell_sorted, minlength=2 * NBLK)
    cell_starts = np.concatenate([[0], np.cumsum(cell_counts)[:-1]])
    rank_in_cell = np.arange(B) - cell_starts[cell_sorted]
    core = rank_in_cell % N_CORES
    pos = rank_in_cell // N_CORES             # slot within (core, cell)
    # per-cell padded size (same for every core)
    nmax = (cell_counts + N_CORES - 1) // N_CORES       # [2*NBLK]
    # bucket sizes in slots, chunk-padded
    b0_slots = int(nmax[:NBLK].sum())
    b1_slots = int(nmax[NBLK:].sum())
    nch = [int(np.ceil(b0_slots / CHUNK)) or 1, int(np.ceil(b1_slots / CHUNK)) or 1]
    ntot = sum(nch) * CHUNK
    # slot offset of each cell in the common layout
    cell_off = np.zeros(2 * NBLK, np.int64)
    cell_off[:NBLK] = np.concatenate([[0], np.cumsum(nmax[:NBLK])[:-1]])
    cell_off[NBLK:] = nch[0] * CHUNK + np.concatenate(
        [[0], np.cumsum(nmax[NBLK:])[:-1]])
    slot = cell_off[cell_sorted] + pos        # per sorted edge

    # ---- per-core inputs ----
    def wrap(a):
        w = np.ascontiguousarray(a.reshape(-1, 16).T.astype(np.int16))
        return np.tile(w, (8, 1))

    onesv = np.ones((128, 1), dtype=ml_dtypes.bfloat16)
    in_maps = []
    placements = []                           # (orig_index, slot) per core
    src_sorted = src[order]
    dst_sorted = dst[order]
    h_sorted = h[order]
    for c in range(N_CORES):
        m = core == c
        sl = slot[m]
        darr = np.zeros(ntot, np.int64)
        darr[sl] = dst_sorted[m] - h_sorted[m] * HALF
        oh = np.zeros((128, ntot), dtype=ml_dtypes.bfloat16)
        oh[src_sorted[m] % 128, sl] = 1
        in_maps.append({"zb": zb, "zbT": zbT, "R": R, "dr": dr, "ones": onesv,
                        "onehot": oh, "idst": wrap(darr)})
        placements.append((order[m], sl))

    # ---- pieces per slice from the common layout ----
    # block id of every slot (-1 for pad/empty slots outside any cell range)
    slot_blk = np.full(ntot, -1, np.int64)
    for cid in range(2 * NBLK):
        n = int(nmax[cid])
        if n:
            slot_blk[cell_off[cid]:cell_off[cid] + n] = cid % NBLK
    # pad slots: attach to block 0 (their one-hot columns are zero, so the
    # matmul writes zeros there and no PSUM region is read unwritten)
    slot_blk[slot_blk < 0] = 0
    pieces = []
    for si in range(ntot // SLICE):
        row = slot_blk[si * SLICE:(si + 1) * SLICE]
        plist = []
        prev = None
        lo = 0
        for e in range(SLICE):
            v = row[e]
            if v != prev:
                if prev is not None:
                    plist.append((int(prev), lo, e))
                prev = v
                lo = e
        plist.append((int(prev), lo, SLICE))
        pieces.append(plist)
    return in_maps, placements, nch, ntot, B, pieces


def _collect(res, placements, B):
    out = np.empty(B, np.float32)
    for c in range(N_CORES):
        sc = np.asarray(res.results[c]["scores"]).reshape(-1)
        orig, sl = placements[c]
        out[orig] = sc[sl]
    return out


def kernel_with_time(inputs, trace=False):
    in_maps, placements, nch, ntot, B, pieces = _prepare(inputs)
    nc = _build_program(nch, pieces)
    res = run_bass_kernel_spmd(nc, in_maps, list(range(N_CORES)), trace=trace)
    out = _collect(res, placements, B)
    return out, res.exec_time_ns, res


def kernel(**inputs):
    out, _, _ = kernel_with_time(inputs, trace=False)
    return out



# revision 14
# speedup vs baseline: 1.3157x; 1.3157x over previous
"""DEDICOM decoder edge scoring on 8 TRN2 NeuronCores.

scores[e] = (z[src_e] * d) @ R @ (z[dst_e] * d)  for 1M edges.

The per-edge row gather is the bottleneck: SWDGE descriptor generation
costs ~2-3 ns/row even with 4 queues pipelining across Q7 core pairs.
So only the DST side uses dma_gather; the SRC side is computed with
TensorE one-hot selection matmuls, which need no descriptors:

  - the host groups edges into (dst-half, src-block) cells and deals
    each cell's edges round-robin across the 8 cores, so every core has
    the IDENTICAL padded layout (all cores share one SPMD program).
    Within a cell, edges are sorted by dst (helps HBM locality of the
    gather).  Host emits, per 512-edge slice, the piece list
    (src_block, lo, hi) plus a one-hot matrix
    onehot[i, slot] = (src_slot mod 128 == i)  (zero for pad slots).
  - device builds Y = z @ M (M = (d (x) d) * R) in bf16, resident in
    SBUF as [128, 391, 128] (Y block b, row r, at [r, b, :]).
  - per 2048-edge chunk: zdT = transpose-mode dma_gather of z[dst]
    (bf16 256B rows -> [128 feat, 2048 edge] columns), round-robin on
    4 SWDGE queues.  Per 512-slice:
      mzs  = Y_b^T @ onehot_piece   (TensorE -> PSUM, per piece)
      prod = mzs * zdT              (DVE, bf16 out)
      sc   = ones^T @ prod          (TensorE -> PSUM [1, 512])
      ACT copies sc to SBUF; Sync DMAs each chunk's scores to DRAM.
  - dst indices are int16, so z is addressed in halves (< / >= 32000).
"""
import numpy as np
import ml_dtypes
import concourse.bacc as bacc
import concourse.mybir as mybir
from concourse.tile import TileContext
from concourse.bass_utils import run_bass_kernel_spmd

N_CORES = 8
N_NODES = 50000
D = 128
HALF = 32000          # int16-safe dst table split point
CHUNK = 4096          # edges per dma_gather call
SLICE = 512           # edges per compute slice (1 PSUM bank)
NQ = 4                # SWDGE queues (queue q -> Q7 core pair q)
NBLK = (N_NODES + 127) // 128             # 391 src blocks
NPAD = NBLK * 128                         # 50048


def _build_program(nchunks_per_bucket, pieces):
    ntot = sum(nchunks_per_bucket) * CHUNK
    nc = bacc.Bacc("TRN2", num_devices=N_CORES, num_swdge_queues=NQ)
    zb = nc.declare_dram_parameter("zb", [NPAD, D], mybir.dt.bfloat16, isOutput=False)
    zbT = nc.declare_dram_parameter("zbT", [128, NPAD], mybir.dt.bfloat16, isOutput=False)
    R = nc.declare_dram_parameter("R", [D, D], mybir.dt.float32, isOutput=False)
    dr = nc.declare_dram_parameter("dr", [1, D], mybir.dt.float32, isOutput=False)
    ones = nc.declare_dram_parameter("ones", [128, 1], mybir.dt.bfloat16, isOutput=False)
    onehot = nc.declare_dram_parameter("onehot", [128, ntot], mybir.dt.bfloat16, isOutput=False)
    idst = nc.declare_dram_parameter("idst", [128, ntot // 16], mybir.dt.int16, isOutput=False)
    scores = nc.declare_dram_parameter("scores", [1, ntot], mybir.dt.float32, isOutput=True)

    with TileContext(nc) as tc:
        with (
            tc.tile_pool(name="const", bufs=1) as constp,
            tc.tile_pool(name="yres", bufs=1) as yresp,
            tc.tile_pool(name="zslab", bufs=1) as zslabp,
            tc.tile_pool(name="ybuild", bufs=2, space="PSUM") as ybps,
            tc.tile_pool(name="idxp", bufs=1) as idxp,
            tc.tile_pool(name="gdst", bufs=3) as gdstp,
            tc.tile_pool(name="zt", bufs=2) as ztp,
            tc.tile_pool(name="ohp", bufs=3) as ohp,
            tc.tile_pool(name="mzs", bufs=3, space="PSUM") as mzsp,
            tc.tile_pool(name="prod", bufs=2) as prodp,
            tc.tile_pool(name="scps", bufs=3, space="PSUM") as scpsp,
            tc.tile_pool(name="scsb", bufs=1) as scsbp,
        ):
            # ---- constants: M = (d (x) d) * R in bf16 ----
            R_sb = constp.tile([128, D], mybir.dt.float32)
            nc.sync.dma_start(out=R_sb[:], in_=R[:])
            dr_sb = constp.tile([1, D], mybir.dt.float32)
            nc.sync.dma_start(out=dr_sb[:], in_=dr[:])
            ones_sb = constp.tile([128, 1], mybir.dt.bfloat16)
            nc.sync.dma_start(out=ones_sb[:], in_=ones[:])
            DRps = ybps.tile([128, 4, 128], mybir.dt.float32, tag="yps")
            nc.tensor.matmul(out=DRps[:, 0, :], lhsT=dr_sb[:], rhs=dr_sb[:],
                             start=True, stop=True)
            Mb = constp.tile([128, D], mybir.dt.bfloat16)
            nc.vector.tensor_tensor(out=Mb[:], in0=R_sb[:], in1=DRps[:, 0, :],
                                    op=mybir.AluOpType.mult)

            # ---- indices ----
            idst_sb = idxp.tile([128, ntot // 16], mybir.dt.int16)
            nc.sync.dma_start(out=idst_sb[:], in_=idst[:])

            # ---- Y = z @ M resident in SBUF, blocked [128, NBLK, 128] ----
            Yres = yresp.tile([128, NBLK, 128], mybir.dt.bfloat16)
            for s0 in range(0, NBLK, 32):          # 32-block slabs of zbT
                sw = min(32, NBLK - s0)
                zsb = zslabp.tile([128, 32 * 128], mybir.dt.bfloat16, tag="zslab")
                nc.sync.dma_start(
                    out=zsb[:, :sw * 128],
                    in_=zbT[:, s0 * 128:(s0 + sw) * 128])
                i = 0
                while i < sw:
                    w = min(4, sw - i)
                    yps = ybps.tile([128, 4, 128], mybir.dt.float32, tag="yps")
                    for j in range(w):
                        nc.tensor.matmul(
                            out=yps[:, j, :],
                            lhsT=zsb[:, (i + j) * 128:(i + j + 1) * 128],
                            rhs=Mb[:], start=True, stop=True)
                    nc.scalar.copy(out=Yres[:, s0 + i:s0 + i + w, :], in_=yps[:, :w, :])
                    i += w

            # ---- main loop ----
            nchunks = ntot // CHUNK
            spc = CHUNK // SLICE
            LA = 3          # oh-load lookahead (chunks)
            oh_tiles = {}

            def load_oh(k):
                oh = ohp.tile([128, CHUNK], mybir.dt.bfloat16, tag="oh")
                nc.scalar.dma_start(out=oh[:], in_=onehot[:, k * CHUNK:(k + 1) * CHUNK])
                oh_tiles[k] = oh

            for k in range(min(LA, ntot // CHUNK)):
                load_oh(k)
            for k in range(nchunks):
                b = 0 if k < nchunks_per_bucket[0] else 1
                dst_t = zb[:, :] if b == 0 else zb[HALF:, :]
                c16 = k * (CHUNK // 16)
                g = gdstp.tile([128, CHUNK // 128, D], mybir.dt.bfloat16, tag="g")
                nc.gpsimd.dma_gather(
                    g[:], dst_t, idst_sb[:, c16:c16 + CHUNK // 16],
                    CHUNK, CHUNK, D, single_packet=False, queue_num=k % NQ)
                # rows -> columns: zdTt[f, c, p] = g[p, c, f]; edge e=(c,p)
                zdTt = ztp.tile([128, CHUNK // 128, D], mybir.dt.bfloat16, tag="zt")
                nc.sync.dma_start_transpose(out=zdTt[:], in_=g[:])
                zdT = zdTt[:, :, :].rearrange("f c p -> f (c p)")
                if k + LA < nchunks:
                    load_oh(k + LA)
                oh = oh_tiles.pop(k)
                sc_sb = scsbp.tile([1, CHUNK], mybir.dt.float32, tag="scsb")
                for j in range(spc):
                    si = k * spc + j
                    mzs = mzsp.tile([128, SLICE], mybir.dt.float32, tag="mzs")
                    for (blk, lo, hi) in pieces[si]:
                        nc.tensor.matmul(
                            out=mzs[:, lo:hi], lhsT=Yres[:, blk, :],
                            rhs=oh[:, j * SLICE + lo:j * SLICE + hi],
                            start=True, stop=True)
                    prod = prodp.tile([128, SLICE], mybir.dt.bfloat16, tag="prod")
                    nc.vector.tensor_tensor(
                        out=prod[:], in0=mzs[:],
                        in1=zdT[:, j * SLICE:(j + 1) * SLICE],
                        op=mybir.AluOpType.mult)
                    scp = scpsp.tile([1, SLICE], mybir.dt.float32, tag="scps")
                    nc.tensor.matmul(out=scp[:], lhsT=ones_sb[:], rhs=prod[:],
                                     start=True, stop=True)
                    nc.scalar.copy(out=sc_sb[:, j * SLICE:(j + 1) * SLICE], in_=scp[:])
                nc.scalar.dma_start(
                    out=scores[:, k * CHUNK:(k + 1) * CHUNK], in_=sc_sb[:])
    nc.compile()
    return nc


def _prepare(inputs):
    z = np.asarray(inputs["z"], dtype=np.float32)
    R = np.ascontiguousarray(np.asarray(inputs["R"], dtype=np.float32))
    Dm = np.asarray(inputs["D"], dtype=np.float32)
    edge_index = np.asarray(inputs["edge_index"])
    rel = int(np.asarray(inputs["relation_idx"]))
    dr = np.ascontiguousarray(Dm[rel:rel + 1, :])
    zb = np.zeros((NPAD, D), dtype=ml_dtypes.bfloat16)
    zb[:N_NODES] = z.astype(ml_dtypes.bfloat16)
    zbT = np.ascontiguousarray(zb.T)

    B = edge_index.shape[1]
    src = edge_index[0].astype(np.int64)
    dst = edge_index[1].astype(np.int64)

    # ---- cell assignment: (dst-half, src-block), dealt across cores ----
    h = (dst >= HALF).astype(np.int64)
    blk = src // 128
    cell = h * NBLK + blk                     # 0 .. 2*NBLK-1
    order = np.lexsort((dst, cell))           # cell-major, dst-sorted inside
    cell_sorted = cell[order]
    # per-cell counts and starts
    cell_counts = np.bincount(cell_sorted, minlength=2 * NBLK)
    cell_starts = np.concatenate([[0], np.cumsum(cell_counts)[:-1]])
    rank_in_cell = np.arange(B) - cell_starts[cell_sorted]
    core = rank_in_cell % N_CORES
    pos = rank_in_cell // N_CORES             # slot within (core, cell)
    # per-cell padded size (same for every core)
    nmax = (cell_counts + N_CORES - 1) // N_CORES       # [2*NBLK]
    # bucket sizes in slots, chunk-padded
    b0_slots = int(nmax[:NBLK].sum())
    b1_slots = int(nmax[NBLK:].sum())
    nch = [int(np.ceil(b0_slots / CHUNK)) or 1, int(np.ceil(b1_slots / CHUNK)) or 1]
    ntot = sum(nch) * CHUNK
    # slot offset of each cell in the common layout
    cell_off = np.zeros(2 * NBLK, np.int64)
    cell_off[:NBLK] = np.concatenate([[0], np.cumsum(nmax[:NBLK])[:-1]])
    cell_off[NBLK:] = nch[0] * CHUNK + np.concatenate(
        [[0], np.cumsum(nmax[NBLK:])[:-1]])
    slot = cell_off[cell_sorted] + pos        # per sorted edge

    # ---- per-core inputs ----
    def wrap(a):
        w = np.ascontiguousarray(a.reshape(-1, 16).T.astype(np.int16))
        return np.tile(w, (8, 1))

    onesv = np.ones((128, 1), dtype=ml_dtypes.bfloat16)
    in_maps = []
    placements = []                           # (orig_index, slot) per core
    src_sorted = src[order]
    dst_sorted = dst[order]
    h_sorted = h[order]
    for c in range(N_CORES):
        m = core == c
        sl = slot[m]
        darr = np.zeros(ntot, np.int64)
        darr[sl] = dst_sorted[m] - h_sorted[m] * HALF
        oh = np.zeros((128, ntot), dtype=ml_dtypes.bfloat16)
        oh[src_sorted[m] % 128, sl] = 1
        in_maps.append({"zb": zb, "zbT": zbT, "R": R, "dr": dr, "ones": onesv,
                        "onehot": oh, "idst": wrap(darr)})
        placements.append((order[m], sl))

    # ---- pieces per slice from the common layout ----
    # block id of every slot (-1 for pad/empty slots outside any cell range)
    slot_blk = np.full(ntot, -1, np.int64)
    for cid in range(2 * NBLK):
        n = int(nmax[cid])
        if n:
            slot_blk[cell_off[cid]:cell_off[cid] + n] = cid % NBLK
    # pad slots: attach to block 0 (their one-hot columns are zero, so the
    # matmul writes zeros there and no PSUM region is read unwritten)
    slot_blk[slot_blk < 0] = 0
    pieces = []
    for si in range(ntot // SLICE):
        row = slot_blk[si * SLICE:(si + 1) * SLICE]
        plist = []
        prev = None
        lo = 0
        for e in range(SLICE):
            v = row[e]
            if v != prev:
                if prev is not None:
                    plist.append((int(prev), lo, e))
                prev = v
                lo = e
        plist.append((int(prev), lo, SLICE))
        pieces.append(plist)
    return in_maps, placements, nch, ntot, B, pieces


def _collect(res, placements, B):
    out = np.empty(B, np.float32)
    for c in range(N_CORES):
        sc = np.asarray(res.results[c]["scores"]).reshape(-1)
        orig, sl = placements[c]
        out[orig] = sc[sl]
    return out


def kernel_with_time(inputs, trace=False):
    in_maps, placements, nch, ntot, B, pieces = _prepare(inputs)
    nc = _build_program(nch, pieces)
    res = run_bass_kernel_spmd(nc, in_maps, list(range(N_CORES)), trace=trace)
    out = _collect(res, placements, B)
    return out, res.exec_time_ns, res


def kernel(**inputs):
    out, _, _ = kernel_with_time(inputs, trace=False)
    return out

